# revision 6
# baseline (speedup 1.0000x reference)
"""CenterLoss (gather + MSE loss + counted scatter-update) on 8 TRN2 NeuronCores.

Strategy (table-parallel, per the sharding hint):
  - The centers table [100000, 256] is sharded row-wise: core k owns rows
    [k*12500, (k+1)*12500).
  - Each sample i belongs to the core owning row target[i]. The host routes
    samples: per core it builds index lists (pure int bookkeeping on `target`,
    no feature math) — which feature rows to gather, which local center rows
    to gather, per-sample 1/(count+eps) scale, and scatter index lists.
  - Duplicate targets are handled with dup-free "rounds": round r holds the
    samples that are the r-th occurrence of their row. Within one
    dma_scatter_add there is then at most one add per destination row, so the
    HBM read-modify-write accumulation has no same-row races. Rounds are
    serialized against each other and against the bulk copy by Tile's
    DRAM-tensor WAW dependency tracking.
  - Per core the device: bulk-copies its centers shard to the output (the
    dominant, unavoidable 2x12.8MB of HBM traffic), gathers its samples'
    feature rows and center rows (dma_gather), computes
    diff = f - c, u = alpha/(n+eps) * diff, per-partition sum of w*diff^2
    (DVE), then scatter-adds u into the output shard (dma_scatter_add).
  - Host assembles: concatenates output shards, sums the 8x128 loss partials
    and divides by B*D.

Slot layout contract (matches InstDMAGatherAnt/InstDMAScatterAddAnt):
  slot j of a gather/scatter lives at SBUF [partition j%128, block j//128];
  index j of an idx list lives at [partition j%16, column j//16], replicated
  8x down the 128 partitions. Pad slots gather row 0 (harmless) and
  scatter-add u=0 (svals=0 on pads) into a dedicated scratch row R, which is
  dropped on the host.
"""

import numpy as np

NUM_CLASSES = 100000
D = 256
B = 16384
ALPHA = 0.5
EPS = 1e-6
M = 8
R = NUM_CLASSES // M  # 12500 rows per shard
BULK_CHUNKS = 4

_cache: dict = {}


def _roundup(x, m):
    return (x + m - 1) // m * m


def _wrap_idx(a):
    """[S] int -> [128, S/16] int16 in the 16-partition wrapped+replicated layout."""
    w = np.ascontiguousarray(a.reshape(-1, 16).T.astype(np.int16))  # [16, S/16]
    return np.ascontiguousarray(np.tile(w, (8, 1)))


def _wrap_slot(a):
    """[S] f32 -> [128, S/128]: slot j -> [j%128, j//128]."""
    return np.ascontiguousarray(a.reshape(-1, 128).T)


def _route(target):
    """Host routing: per-core sorted sample lists + dup-free round structure."""
    tgt = np.asarray(target).astype(np.int64)
    counts = np.bincount(tgt, minlength=NUM_CLASSES)
    owner = tgt // R
    local = tgt % R

    cores = []
    max_rank = 0
    for k in range(M):
        I = np.nonzero(owner == k)[0]
        loc = local[I]
        order = np.argsort(loc, kind="stable")
        I_s, loc_s = I[order], loc[order]
        if len(loc_s):
            newgrp = np.concatenate([[True], loc_s[1:] != loc_s[:-1]])
            grp_start = np.maximum.accumulate(
                np.where(newgrp, np.arange(len(loc_s)), 0)
            )
            ranks = np.arange(len(loc_s)) - grp_start
            max_rank = max(max_rank, int(ranks.max()))
        else:
            ranks = np.zeros(0, np.int64)
        cores.append((I_s, loc_s, ranks))

    n_rounds = max_rank + 1
    caps = []
    for r in range(n_rounds):
        mx = max(int((ranks == r).sum()) for _, _, ranks in cores)
        caps.append(max(_roundup(mx, 128), 128))
    return cores, tuple(caps), counts, tgt


def _core_arrays(core, caps, counts, tgt):
    I_s, loc_s, ranks = core
    S = sum(caps)
    feat_idx = np.zeros(S, np.int64)
    cent_idx = np.zeros(S, np.int64)
    svals = np.zeros(S, np.float32)
    wvals = np.zeros(S, np.float32)
    scat = []
    off = 0
    for r, cap in enumerate(caps):
        sel = ranks == r
        m = int(sel.sum())
        feat_idx[off:off + m] = I_s[sel]
        cent_idx[off:off + m] = loc_s[sel]
        si = np.full(cap, R, np.int64)  # pads -> scratch row R
        si[:m] = loc_s[sel]
        n = counts[tgt[I_s[sel]]]
        svals[off:off + m] = (ALPHA / (n.astype(np.float64) + EPS)).astype(np.float32)
        wvals[off:off + m] = 1.0
        scat.append(si)
        off += cap
    return {
        "feat_idx": _wrap_idx(feat_idx),
        "cent_idx": _wrap_idx(cent_idx),
        "svals": _wrap_slot(svals),
        "wvals": _wrap_slot(wvals),
        **{f"scat_idx_{r}": _wrap_idx(s) for r, s in enumerate(scat)},
    }


def _build(caps):
    """Build + compile the SPMD Bass program for a given round structure."""
    key = ("v1", caps)
    if key in _cache:
        return _cache[key]

    import concourse.bacc as bacc
    import concourse.mybir as mybir
    import concourse.tile as tile

    dt = mybir.dt
    S = sum(caps)
    NB = S // 128

    nc = bacc.Bacc("TRN2", target_bir_lowering=False)
    centers_d = nc.dram_tensor("centers_shard", [R, D], dt.float32,
                               kind="ExternalInput")
    features_d = nc.dram_tensor("features", [B, D], dt.float32,
                                kind="ExternalInput")
    fidx_d = nc.dram_tensor("feat_idx", [128, S // 16], dt.int16,
                            kind="ExternalInput")
    cidx_d = nc.dram_tensor("cent_idx", [128, S // 16], dt.int16,
                            kind="ExternalInput")
    sv_d = nc.dram_tensor("svals", [128, NB], dt.float32, kind="ExternalInput")
    wv_d = nc.dram_tensor("wvals", [128, NB], dt.float32, kind="ExternalInput")
    sidx_d = [
        nc.dram_tensor(f"scat_idx_{r}", [128, cap // 16], dt.int16,
                       kind="ExternalInput")
        for r, cap in enumerate(caps)
    ]
    out_d = nc.dram_tensor("new_centers", [R + 1, D], dt.float32,
                           kind="ExternalOutput")
    loss_d = nc.dram_tensor("loss_part", [128, 1], dt.float32,
                            kind="ExternalOutput")

    mult = mybir.AluOpType.mult

    with tile.TileContext(nc) as tc:
        with tc.tile_pool(name="p", bufs=1) as pool:
            fidx = pool.tile([128, S // 16], dt.int16)
            cidx = pool.tile([128, S // 16], dt.int16)
            sidx = [
                pool.tile([128, cap // 16], dt.int16, name=f"sidx{r}")
                for r, cap in enumerate(caps)
            ]
            sv = pool.tile([128, NB], dt.float32)
            wv = pool.tile([128, NB], dt.float32)
            fg = pool.tile([128, NB, D], dt.float32)
            cg = pool.tile([128, NB, D], dt.float32)
            diff = pool.tile([128, NB, D], dt.float32)
            u = pool.tile([128, NB, D], dt.float32)
            lw = pool.tile([128, NB, D], dt.float32)
            lpart = pool.tile([128, NB], dt.float32)
            lsum = pool.tile([128, 1], dt.float32)

            # metadata loads
            nc.sync.dma_start(fidx[:], fidx_d[:, :])
            nc.sync.dma_start(cidx[:], cidx_d[:, :])
            for r in range(len(caps)):
                nc.sync.dma_start(sidx[r][:], sidx_d[r][:, :])
            nc.sync.dma_start(sv[:], sv_d[:, :])
            nc.sync.dma_start(wv[:], wv_d[:, :])

            # gathers (SWDGE): features rows + this shard's center rows.
            # single_packet=True breaks beyond ~1K descriptors (HW packet
            # limit) -> False for the big gathers.
            nc.gpsimd.dma_gather(fg[:], features_d[:, :], fidx[:], S, S, D,
                                 single_packet=False)
            nc.gpsimd.dma_gather(cg[:], centers_d[:, :], cidx[:], S, S, D,
                                 single_packet=False)

            # bulk shard copy HBM->HBM (the mandatory 2x table traffic)
            rows = R // BULK_CHUNKS
            for c in range(BULK_CHUNKS):
                nc.sync.dma_start(
                    out_d[c * rows:(c + 1) * rows, :],
                    centers_d[c * rows:(c + 1) * rows, :],
                )

            # DVE: diff = f - c; u = s*diff; lw = (w*diff)*diff with
            # per-partition accumulation into lpart
            for b in range(NB):
                nc.vector.tensor_sub(diff[:, b, :], fg[:, b, :], cg[:, b, :])
                nc.vector.tensor_scalar_mul(u[:, b, :], diff[:, b, :],
                                            sv[:, b:b + 1])
                nc.vector.scalar_tensor_tensor(
                    lw[:, b, :], diff[:, b, :], wv[:, b:b + 1], diff[:, b, :],
                    op0=mult, op1=mult, accum_out=lpart[:, b:b + 1],
                )
            nc.vector.reduce_sum(lsum[:], lpart[:], axis=mybir.AxisListType.X)
            nc.sync.dma_start(loss_d[:, :], lsum[:])

            # dup-free scatter-add rounds into the output shard
            off = 0
            for r, cap in enumerate(caps):
                nb = cap // 128
                nc.gpsimd.dma_scatter_add(
                    out_d[:, :], u[:, off:off + nb, :], sidx[r][:], cap, cap, D,
                    single_packet=(cap <= 256),
                )
                off += nb

    nc.compile()
    _cache[key] = nc
    return nc


def kernel(centers, features, target):
    from concourse import bass_utils

    centers = np.ascontiguousarray(np.asarray(centers, dtype=np.float32))
    features = np.ascontiguousarray(np.asarray(features, dtype=np.float32))
    tgt_in = np.asarray(target)

    cores, caps, counts, tgt = _route(tgt_in)
    nc = _build(caps)

    in_maps = []
    for k in range(M):
        arrays = _core_arrays(cores[k], caps, counts, tgt)
        in_maps.append({
            "centers_shard": np.ascontiguousarray(centers[k * R:(k + 1) * R]),
            "features": features,
            **arrays,
        })

    import os

    trace = os.environ.get("KERNEL_TRACE", "0") == "1"
    res = bass_utils.run_bass_kernel_spmd(
        nc, in_maps, core_ids=list(range(M)), trace=trace
    )
    globals()["last_result"] = res

    shards = [res.results[k]["new_centers"][:R] for k in range(M)]
    new_centers = np.concatenate(shards, axis=0)
    total = np.float64(0.0)
    for k in range(M):
        total += np.float64(res.results[k]["loss_part"].sum(dtype=np.float64))
    loss = np.float32(total / (B * D))
    return loss, new_centers


# revision 8
# speedup vs baseline: 2.5950x; 2.5950x over previous
"""CenterLoss (gather + MSE loss + counted scatter-update) on 8 TRN2 NeuronCores.

Strategy (table-parallel, per the sharding hint):
  - The centers table [100000, 256] is sharded row-wise: core k owns rows
    [k*12500, (k+1)*12500).
  - Each sample i belongs to the core owning row target[i]. The host routes
    samples: per core it builds index lists (pure int bookkeeping on `target`,
    no feature math) — which feature rows to gather, which local center rows
    to gather, per-sample 1/(count+eps) scale, and scatter index lists.
  - Duplicate targets are handled with dup-free "rounds": round r holds the
    samples that are the r-th occurrence of their row. Within one
    dma_scatter_add there is then at most one add per destination row, so the
    HBM read-modify-write accumulation has no same-row races. Rounds are
    serialized against each other and against the bulk copy by Tile's
    DRAM-tensor WAW dependency tracking.
  - Per core the device: bulk-copies its centers shard to the output (the
    dominant, unavoidable 2x12.8MB of HBM traffic), gathers its samples'
    feature rows and center rows (dma_gather), computes
    diff = f - c, u = alpha/(n+eps) * diff, per-partition sum of w*diff^2
    (DVE), then scatter-adds u into the output shard (dma_scatter_add).
  - Host assembles: concatenates output shards, sums the 8x128 loss partials
    and divides by B*D.

Slot layout contract (matches InstDMAGatherAnt/InstDMAScatterAddAnt):
  slot j of a gather/scatter lives at SBUF [partition j%128, block j//128];
  index j of an idx list lives at [partition j%16, column j//16], replicated
  8x down the 128 partitions. Pad slots gather row 0 (harmless) and
  scatter-add u=0 (svals=0 on pads) into a dedicated scratch row R, which is
  dropped on the host.
"""

import numpy as np

NUM_CLASSES = 100000
D = 256
B = 16384
ALPHA = 0.5
EPS = 1e-6
M = 8
R = NUM_CLASSES // M  # 12500 rows per shard
BULK_CHUNKS = 4

_cache: dict = {}


def _roundup(x, m):
    return (x + m - 1) // m * m


def _wrap_idx(a):
    """[S] int -> [128, S/16] int16 in the 16-partition wrapped+replicated layout."""
    w = np.ascontiguousarray(a.reshape(-1, 16).T.astype(np.int16))  # [16, S/16]
    return np.ascontiguousarray(np.tile(w, (8, 1)))


def _wrap_slot(a):
    """[S] f32 -> [128, S/128]: slot j -> [j%128, j//128]."""
    return np.ascontiguousarray(a.reshape(-1, 128).T)


def _route(target):
    """Host routing: per-core sorted sample lists + dup-free round structure."""
    tgt = np.asarray(target).astype(np.int64)
    counts = np.bincount(tgt, minlength=NUM_CLASSES)
    owner = tgt // R
    local = tgt % R

    cores = []
    max_rank = 0
    for k in range(M):
        I = np.nonzero(owner == k)[0]
        loc = local[I]
        order = np.argsort(loc, kind="stable")
        I_s, loc_s = I[order], loc[order]
        if len(loc_s):
            newgrp = np.concatenate([[True], loc_s[1:] != loc_s[:-1]])
            grp_start = np.maximum.accumulate(
                np.where(newgrp, np.arange(len(loc_s)), 0)
            )
            ranks = np.arange(len(loc_s)) - grp_start
            max_rank = max(max_rank, int(ranks.max()))
        else:
            ranks = np.zeros(0, np.int64)
        cores.append((I_s, loc_s, ranks))

    n_rounds = max_rank + 1
    caps = []
    for r in range(n_rounds):
        mx = max(int((ranks == r).sum()) for _, _, ranks in cores)
        caps.append(max(_roundup(mx, 128), 128))
    return cores, tuple(caps), counts, tgt


def _core_arrays(core, caps, counts, tgt):
    I_s, loc_s, ranks = core
    S = sum(caps)
    feat_idx = np.zeros(S, np.int64)
    cent_idx = np.zeros(S, np.int64)
    svals = np.zeros(S, np.float32)
    wvals = np.zeros(S, np.float32)
    scat = []
    off = 0
    for r, cap in enumerate(caps):
        sel = ranks == r
        m = int(sel.sum())
        feat_idx[off:off + m] = I_s[sel]
        cent_idx[off:off + m] = loc_s[sel]
        si = np.full(cap, R, np.int64)  # pads -> scratch row R
        si[:m] = loc_s[sel]
        n = counts[tgt[I_s[sel]]]
        svals[off:off + m] = (ALPHA / (n.astype(np.float64) + EPS)).astype(np.float32)
        wvals[off:off + m] = 1.0
        scat.append(si)
        off += cap
    return {
        "feat_idx": _wrap_idx(feat_idx),
        "cent_idx": _wrap_idx(cent_idx),
        "svals": _wrap_slot(svals),
        "wvals": _wrap_slot(wvals),
        **{f"scat_idx_{r}": _wrap_idx(s) for r, s in enumerate(scat)},
    }


def _build(caps, repeat=1, barrier=False):
    """Build + compile the SPMD Bass program for a given round structure.

    repeat>1 emits the whole body K times inside one NEFF (timing builds:
    K-slope wall-clock measurement amortizes the per-launch overhead).
    """
    key = ("v1", caps, repeat, barrier)
    if key in _cache:
        return _cache[key]

    import concourse.bacc as bacc
    import concourse.mybir as mybir
    import concourse.tile as tile

    dt = mybir.dt
    S = sum(caps)
    NB = S // 128

    nc = bacc.Bacc("TRN2", target_bir_lowering=False)
    centers_d = nc.dram_tensor("centers_shard", [R, D], dt.float32,
                               kind="ExternalInput")
    features_d = nc.dram_tensor("features", [B, D], dt.float32,
                                kind="ExternalInput")
    fidx_d = nc.dram_tensor("feat_idx", [128, S // 16], dt.int16,
                            kind="ExternalInput")
    cidx_d = nc.dram_tensor("cent_idx", [128, S // 16], dt.int16,
                            kind="ExternalInput")
    sv_d = nc.dram_tensor("svals", [128, NB], dt.float32, kind="ExternalInput")
    wv_d = nc.dram_tensor("wvals", [128, NB], dt.float32, kind="ExternalInput")
    sidx_d = [
        nc.dram_tensor(f"scat_idx_{r}", [128, cap // 16], dt.int16,
                       kind="ExternalInput")
        for r, cap in enumerate(caps)
    ]
    out_d = nc.dram_tensor("new_centers", [R + 1, D], dt.float32,
                           kind="ExternalOutput")
    loss_d = nc.dram_tensor("loss_part", [128, 1], dt.float32,
                            kind="ExternalOutput")

    mult = mybir.AluOpType.mult

    with tile.TileContext(nc) as tc:
        with tc.tile_pool(name="p", bufs=1) as pool:
            fidx = pool.tile([128, S // 16], dt.int16)
            cidx = pool.tile([128, S // 16], dt.int16)
            sidx = [
                pool.tile([128, cap // 16], dt.int16, name=f"sidx{r}")
                for r, cap in enumerate(caps)
            ]
            sv = pool.tile([128, NB], dt.float32)
            wv = pool.tile([128, NB], dt.float32)
            fg = pool.tile([128, NB, D], dt.float32)
            cg = pool.tile([128, NB, D], dt.float32)
            diff = pool.tile([128, NB, D], dt.float32)
            u = pool.tile([128, NB, D], dt.float32)
            lw = pool.tile([128, NB, D], dt.float32)
            lpart = pool.tile([128, NB], dt.float32)
            lsum = pool.tile([128, 1], dt.float32)

            # metadata loads
            nc.sync.dma_start(fidx[:], fidx_d[:, :])
            nc.sync.dma_start(cidx[:], cidx_d[:, :])
            for r in range(len(caps)):
                nc.sync.dma_start(sidx[r][:], sidx_d[r][:, :])
            nc.sync.dma_start(sv[:], sv_d[:, :])
            nc.sync.dma_start(wv[:], wv_d[:, :])

            for it in range(repeat):
                # gathers (SWDGE): features rows + this shard's center rows.
                # single_packet=True breaks beyond ~1K descriptors (HW packet
                # limit) -> False for the big gathers.
                nc.gpsimd.dma_gather(fg[:], features_d[:, :], fidx[:], S, S, D,
                                     single_packet=False)
                nc.gpsimd.dma_gather(cg[:], centers_d[:, :], cidx[:], S, S, D,
                                     single_packet=False)

                # bulk shard copy HBM->HBM (the mandatory 2x table traffic)
                rows = R // BULK_CHUNKS
                for c in range(BULK_CHUNKS):
                    nc.sync.dma_start(
                        out_d[c * rows:(c + 1) * rows, :],
                        centers_d[c * rows:(c + 1) * rows, :],
                    )

                # DVE: diff = f - c; u = s*diff; lw = (w*diff)*diff with
                # per-partition accumulation into lpart
                for b in range(NB):
                    nc.vector.tensor_sub(diff[:, b, :], fg[:, b, :], cg[:, b, :])
                    nc.vector.tensor_scalar_mul(u[:, b, :], diff[:, b, :],
                                                sv[:, b:b + 1])
                    nc.vector.scalar_tensor_tensor(
                        lw[:, b, :], diff[:, b, :], wv[:, b:b + 1], diff[:, b, :],
                        op0=mult, op1=mult, accum_out=lpart[:, b:b + 1],
                    )
                nc.vector.reduce_sum(lsum[:], lpart[:], axis=mybir.AxisListType.X)
                nc.sync.dma_start(loss_d[:, :], lsum[:])

                # dup-free scatter-add rounds into the output shard
                off = 0
                for r, cap in enumerate(caps):
                    nb = cap // 128
                    nc.gpsimd.dma_scatter_add(
                        out_d[:, :], u[:, off:off + nb, :], sidx[r][:], cap, cap,
                        D, single_packet=(cap <= 256),
                    )
                    off += nb
                if barrier and it != repeat - 1:
                    tc.strict_bb_all_engine_barrier()

    nc.compile()
    _cache[key] = nc
    return nc


def kernel(centers, features, target):
    from concourse import bass_utils

    centers = np.ascontiguousarray(np.asarray(centers, dtype=np.float32))
    features = np.ascontiguousarray(np.asarray(features, dtype=np.float32))
    tgt_in = np.asarray(target)

    cores, caps, counts, tgt = _route(tgt_in)
    nc = _build(caps)

    in_maps = []
    for k in range(M):
        arrays = _core_arrays(cores[k], caps, counts, tgt)
        in_maps.append({
            "centers_shard": np.ascontiguousarray(centers[k * R:(k + 1) * R]),
            "features": features,
            **arrays,
        })

    import os

    trace = os.environ.get("KERNEL_TRACE", "0") == "1"
    res = bass_utils.run_bass_kernel_spmd(
        nc, in_maps, core_ids=list(range(M)), trace=trace
    )
    globals()["last_result"] = res

    shards = [res.results[k]["new_centers"][:R] for k in range(M)]
    new_centers = np.concatenate(shards, axis=0)
    total = np.float64(0.0)
    for k in range(M):
        total += np.float64(res.results[k]["loss_part"].sum(dtype=np.float64))
    loss = np.float32(total / (B * D))
    return loss, new_centers


# revision 10
# speedup vs baseline: 3.7969x; 1.4632x over previous
"""CenterLoss (gather + MSE loss + counted scatter-update) on 8 TRN2 NeuronCores.

Strategy (table-parallel, per the sharding hint):
  - The centers table [100000, 256] is sharded row-wise: core k owns rows
    [k*12500, (k+1)*12500). Each sample i is routed (host-side int
    bookkeeping on `target` only) to the core owning row target[i].
  - Per core the device:
      * bulk-copies its centers shard to the output (the mandatory 2x12.8MB
        of HBM traffic),
      * dma_gathers its samples' feature rows and center rows,
      * computes diff = f - c, u = alpha/(n+eps) * diff and the per-partition
        loss partials sum(w * diff^2) on DVE,
      * pre-combines duplicate-target samples' u rows with one-hot PE matmuls
        accumulated in PSUM (so every destination row has exactly ONE update),
      * dma_scatter_adds all updates in a single dup-free instruction.
  - Host assembles: concatenates output shards (dropping each shard's scratch
    row), sums the 8x128 loss partials and divides by B*D.

Slot layout contract (matches InstDMAGatherAnt/InstDMAScatterAddAnt):
  slot j of a gather/scatter lives at SBUF [partition j%128, block j//128];
  index j of an idx list lives at [partition j%16, column j//16], replicated
  8x down the 128 partitions.

Slot groups (each 128-padded; caps shared across cores = max over cores):
  A: samples whose target row appears once in the batch  -> scattered as-is
  B: samples of rows appearing >=2 times (sorted by row) -> matmul-combined
  G: the unique rows of B; combined updates land in scatter slots [A | G]
Pad slots gather row 0 (harmless), carry svals=wvals=0 (=> u=0, no loss), and
scatter into a dedicated scratch row R that the host drops.
"""

import numpy as np

NUM_CLASSES = 100000
D = 256
B = 16384
ALPHA = 0.5
EPS = 1e-6
M = 8
R = NUM_CLASSES // M   # 12500 rows per shard
BULK_CHUNKS = 4

_cache: dict = {}


def _roundup(x, m):
    return (x + m - 1) // m * m


def _wrap_idx(a):
    """[S] int -> [128, S/16] int16 in the 16-partition wrapped+replicated layout."""
    w = np.ascontiguousarray(a.reshape(-1, 16).T.astype(np.int16))  # [16, S/16]
    return np.ascontiguousarray(np.tile(w, (8, 1)))


def _wrap_slot(a):
    """[S] f32 -> [128, S/128]: slot j -> [j%128, j//128]."""
    return np.ascontiguousarray(a.reshape(-1, 128).T)


def _route(target):
    """Host routing: per-core single/multi sample groups + shared caps."""
    tgt = np.asarray(target).astype(np.int64)
    counts = np.bincount(tgt, minlength=NUM_CLASSES)
    owner = tgt // R
    local = tgt % R

    cores = []
    mxA = mxB = mxG = 0
    for k in range(M):
        I = np.nonzero(owner == k)[0]
        loc = local[I]
        order = np.argsort(loc, kind="stable")
        I_s, loc_s = I[order], loc[order]
        multi = counts[tgt[I_s]] >= 2
        A_I = I_s[~multi]
        A_loc = loc_s[~multi]
        B_I = I_s[multi]           # sorted by row (loc_s sorted)
        B_loc = loc_s[multi]
        G_loc = np.unique(B_loc)   # ascending unique rows
        cores.append((A_I, A_loc, B_I, B_loc, G_loc))
        mxA = max(mxA, len(A_I))
        mxB = max(mxB, len(B_I))
        mxG = max(mxG, len(G_loc))

    caps = (
        max(_roundup(mxA, 128), 128),
        max(_roundup(mxB, 128), 128),
        max(_roundup(mxG, 128), 128),
    )
    return cores, caps, counts, tgt


def _core_arrays(core, caps, counts, tgt):
    A_I, A_loc, B_I, B_loc, G_loc = core
    capA, capB, capG = caps
    S = capA + capB          # gather slots
    nA, nB, nG = len(A_I), len(B_I), len(G_loc)

    feat_idx = np.zeros(S, np.int64)
    cent_idx = np.zeros(S, np.int64)
    svals = np.zeros(S, np.float32)
    wvals = np.zeros(S, np.float32)

    feat_idx[:nA] = A_I
    cent_idx[:nA] = A_loc
    nAll = counts[tgt[A_I]]
    svals[:nA] = (ALPHA / (nAll.astype(np.float64) + EPS)).astype(np.float32)
    wvals[:nA] = 1.0
    feat_idx[capA:capA + nB] = B_I
    cent_idx[capA:capA + nB] = B_loc
    nB_counts = counts[tgt[B_I]]
    svals[capA:capA + nB] = (
        ALPHA / (nB_counts.astype(np.float64) + EPS)
    ).astype(np.float32)
    wvals[capA:capA + nB] = 1.0

    # one-hot combine matrix: M[j, g] = 1 iff B sample j targets G row g.
    # SBUF layout [128, Bblk * capG]: mm[p, bb*capG + g] = M[bb*128+p, g]
    Bblk = capB // 128
    mm = np.zeros((128, Bblk * capG), np.float32)
    g_of = np.searchsorted(G_loc, B_loc)
    for j in range(nB):
        mm[j % 128, (j // 128) * capG + g_of[j]] = 1.0

    # scatter: slots [A | G], pads -> scratch row R
    scat = np.full(capA + capG, R, np.int64)
    scat[:nA] = A_loc
    scat[capA:capA + nG] = G_loc

    return {
        "feat_idx": _wrap_idx(feat_idx),
        "cent_idx": _wrap_idx(cent_idx),
        "svals": _wrap_slot(svals),
        "wvals": _wrap_slot(wvals),
        "mmat": mm,
        "scat_idx": _wrap_idx(scat),
    }


def _build(caps, repeat=1, barrier=False):
    """Build + compile the SPMD Bass program for the given caps.

    repeat>1 emits the whole body K times inside one NEFF (timing builds:
    K-slope wall-clock measurement amortizes the per-launch overhead).
    """
    key = ("v3", caps, repeat, barrier)
    if key in _cache:
        return _cache[key]

    import concourse.bacc as bacc
    import concourse.mybir as mybir
    import concourse.tile as tile

    dt = mybir.dt
    capA, capB, capG = caps
    S = capA + capB
    NB = S // 128
    Ablk, Bblk, Gblk = capA // 128, capB // 128, capG // 128
    Sscat = capA + capG

    nc = bacc.Bacc("TRN2", target_bir_lowering=False)
    centers_d = nc.dram_tensor("centers_shard", [R, D], dt.float32,
                               kind="ExternalInput")
    features_d = nc.dram_tensor("features", [B, D], dt.float32,
                                kind="ExternalInput")
    fidx_d = nc.dram_tensor("feat_idx", [128, S // 16], dt.int16,
                            kind="ExternalInput")
    cidx_d = nc.dram_tensor("cent_idx", [128, S // 16], dt.int16,
                            kind="ExternalInput")
    sv_d = nc.dram_tensor("svals", [128, NB], dt.float32, kind="ExternalInput")
    wv_d = nc.dram_tensor("wvals", [128, NB], dt.float32, kind="ExternalInput")
    mm_d = nc.dram_tensor("mmat", [128, Bblk * capG], dt.float32,
                          kind="ExternalInput")
    sidx_d = nc.dram_tensor("scat_idx", [128, Sscat // 16], dt.int16,
                            kind="ExternalInput")
    out_d = nc.dram_tensor("new_centers", [R + 1, D], dt.float32,
                           kind="ExternalOutput")
    loss_d = nc.dram_tensor("loss_part", [128, 1], dt.float32,
                            kind="ExternalOutput")

    mult = mybir.AluOpType.mult

    with tile.TileContext(nc) as tc:
        with (
            tc.tile_pool(name="p", bufs=1) as pool,
            tc.tile_pool(name="ps", bufs=1, space="PSUM") as psum,
        ):
            fidx = pool.tile([128, S // 16], dt.int16)
            cidx = pool.tile([128, S // 16], dt.int16)
            sidx = pool.tile([128, Sscat // 16], dt.int16)
            sv = pool.tile([128, NB], dt.float32)
            wv = pool.tile([128, NB], dt.float32)
            mm = pool.tile([128, Bblk * capG], dt.float32)
            fg = pool.tile([128, NB, D], dt.float32)
            cg = pool.tile([128, NB, D], dt.float32)
            diff = pool.tile([128, NB, D], dt.float32)
            uscat = pool.tile([128, Ablk + Gblk, D], dt.float32)
            uB = pool.tile([128, Bblk, D], dt.float32)
            lw = pool.tile([128, NB, D], dt.float32)
            lpart = pool.tile([128, NB], dt.float32)
            lsum = pool.tile([128, 1], dt.float32)
            pacc = [
                psum.tile([128, D], dt.float32, name=f"pacc{s}")
                for s in range(Gblk)
            ]

            # metadata loads
            nc.sync.dma_start(fidx[:], fidx_d[:, :])
            nc.sync.dma_start(cidx[:], cidx_d[:, :])
            nc.sync.dma_start(sidx[:], sidx_d[:, :])
            nc.sync.dma_start(sv[:], sv_d[:, :])
            nc.sync.dma_start(wv[:], wv_d[:, :])
            nc.sync.dma_start(mm[:], mm_d[:, :])

            for it in range(repeat):
                # gathers (SWDGE): features rows + this shard's center rows.
                # single_packet=True breaks beyond ~1K descriptors -> False.
                nc.gpsimd.dma_gather(fg[:], features_d[:, :], fidx[:], S, S, D,
                                     single_packet=False)
                nc.gpsimd.dma_gather(cg[:], centers_d[:, :], cidx[:], S, S, D,
                                     single_packet=False)

                # bulk shard copy HBM->HBM (the mandatory 2x table traffic)
                rows = R // BULK_CHUNKS
                for c in range(BULK_CHUNKS):
                    nc.sync.dma_start(
                        out_d[c * rows:(c + 1) * rows, :],
                        centers_d[c * rows:(c + 1) * rows, :],
                    )

                # DVE: diff = f - c; u = s*diff (A slots -> uscat, B -> uB);
                # lw = (w*diff)*diff with per-partition accum into lpart
                for b in range(NB):
                    udst = uscat[:, b, :] if b < Ablk else uB[:, b - Ablk, :]
                    nc.vector.tensor_sub(diff[:, b, :], fg[:, b, :], cg[:, b, :])
                    nc.vector.tensor_scalar_mul(udst, diff[:, b, :],
                                                sv[:, b:b + 1])
                    nc.vector.scalar_tensor_tensor(
                        lw[:, b, :], diff[:, b, :], wv[:, b:b + 1], diff[:, b, :],
                        op0=mult, op1=mult, accum_out=lpart[:, b:b + 1],
                    )
                nc.vector.reduce_sum(lsum[:], lpart[:], axis=mybir.AxisListType.X)
                nc.sync.dma_start(loss_d[:, :], lsum[:])

                # combine duplicate rows: pacc[s][g, :] = sum_j M[j, s*128+g] * uB[j, :]
                for s in range(Gblk):
                    for bb in range(Bblk):
                        nc.tensor.matmul(
                            pacc[s][:, :],
                            mm[:, bb * capG + s * 128: bb * capG + (s + 1) * 128],
                            uB[:, bb, :],
                            start=(bb == 0), stop=(bb == Bblk - 1),
                        )
                    nc.vector.tensor_copy(uscat[:, Ablk + s, :], pacc[s][:, :])

                # single dup-free scatter-add of all updates
                nc.gpsimd.dma_scatter_add(
                    out_d[:, :], uscat[:], sidx[:], Sscat, Sscat, D,
                    single_packet=False,
                )
                if barrier and it != repeat - 1:
                    tc.strict_bb_all_engine_barrier()

    nc.compile()
    _cache[key] = nc
    return nc


def kernel(centers, features, target):
    import os

    from concourse import bass_utils

    centers = np.ascontiguousarray(np.asarray(centers, dtype=np.float32))
    features = np.ascontiguousarray(np.asarray(features, dtype=np.float32))
    tgt_in = np.asarray(target)

    cores, caps, counts, tgt = _route(tgt_in)
    nc = _build(caps)

    in_maps = []
    for k in range(M):
        arrays = _core_arrays(cores[k], caps, counts, tgt)
        in_maps.append({
            "centers_shard": np.ascontiguousarray(centers[k * R:(k + 1) * R]),
            "features": features,
            **arrays,
        })

    trace = os.environ.get("KERNEL_TRACE", "0") == "1"
    res = bass_utils.run_bass_kernel_spmd(
        nc, in_maps, core_ids=list(range(M)), trace=trace
    )
    globals()["last_result"] = res

    shards = [res.results[k]["new_centers"][:R] for k in range(M)]
    new_centers = np.concatenate(shards, axis=0)
    total = np.float64(0.0)
    for k in range(M):
        total += np.float64(res.results[k]["loss_part"].sum(dtype=np.float64))
    loss = np.float32(total / (B * D))
    return loss, new_centers
